# revision 1
# baseline (speedup 1.0000x reference)
"""Trainium2 Bass kernel for nn_LinearSEM.

Reference computes: z = solve_triangular(I - strict_lower(tril(w*mask)), (x*diag)^T).T
Algebraic reformulation: z = x @ W_eff with W_eff = diag(d) @ inv(I-L)^T —
the tiny 128x128 unit-lower-triangular inverse is computed on host in float64
(forward substitution, exact structure, no pivoting noise), and the device
kernel becomes a pure streaming GEMM, which is HBM-bandwidth-bound.

Sharding: data-parallel over batch across 8 cores. Each core receives its
x-shard pre-transposed ([128 vars, 65536 batch]) so the contraction dim (vars)
lands on SBUF partitions; W_eff is the PE-stationary operand (loaded once) and
x^T streams through 512 columns per matmul producing z^T, which is stored
contiguously and un-transposed on host.
"""

import numpy as np

NUM_VARS = 128
BATCH = 524288
N_CORES = 8
SHARD = BATCH // N_CORES  # 65536
DMA_TILE = 2048           # fp32 batch-cols per in-DMA tile: 128p x 8KB = 1 MiB/transfer
MM_N = 512                # max fp32 moving free dim per matmul


def _w_eff(weight: np.ndarray, mask: np.ndarray) -> np.ndarray:
    n = NUM_VARS
    wl = np.tril(weight.astype(np.float64) * mask.astype(np.float64))
    d = np.diag(wl).copy()
    L = wl - np.diag(d)
    # X = inv(I - L) by forward substitution in float64: X[i,:] = e_i + L[i,:i] @ X[:i,:]
    X = np.eye(n, dtype=np.float64)
    for i in range(1, n):
        X[i, :] += L[i, :i] @ X[:i, :]
    w_eff = d[:, None] * X.T
    return np.ascontiguousarray(w_eff.astype(np.float32))


def _build_bass(
    dma_tile=DMA_TILE,
    mm_n=MM_N,
    out_chunk=512,    # cols per output z tile / out-DMA; default = dma_tile
    xbufs=5,
    zbufs=12,
    pbufs=8,
    do_mm=True,
    do_copy=True,
    do_out=True,
    copy_engines="v",   # 'a'=ACT only, 'v'=DVE only, 'av'=alternate
    copy_split=False,   # split each PSUM->SBUF copy in half across ACT+DVE
    out_on_act=True,    # issue out-DMAs on the ACT HWDGE queue (separate FIFO)
    reps=1,             # repeat the whole sweep (for slope-based HW timing)
):
    import concourse.bacc as bacc
    import concourse.mybir as mybir
    from concourse.tile import TileContext

    if out_chunk is None:
        out_chunk = dma_tile
    assert dma_tile % out_chunk == 0 and out_chunk % mm_n == 0

    nc = bacc.Bacc(None, target_bir_lowering=False)
    xt = nc.dram_tensor("xt", [NUM_VARS, SHARD], mybir.dt.float32, kind="ExternalInput")
    w = nc.dram_tensor("w", [NUM_VARS, NUM_VARS], mybir.dt.float32, kind="ExternalInput")
    zt = nc.dram_tensor("zt", [NUM_VARS, SHARD], mybir.dt.float32, kind="ExternalOutput")

    with TileContext(nc) as tc:
        with (
            tc.tile_pool(name="wp", bufs=1) as wp,
            tc.tile_pool(name="xp", bufs=xbufs) as xp,
            tc.tile_pool(name="zp", bufs=zbufs) as zp,
            tc.tile_pool(name="pp", bufs=pbufs, space="PSUM") as pp,
        ):
            w_sb = wp.tile([NUM_VARS, NUM_VARS], mybir.dt.float32)
            nc.sync.dma_start(w_sb[:], w[:])
            nmm = 0
            for t in range(reps * (SHARD // dma_tile)):
                t = t % (SHARD // dma_tile)
                x_sb = xp.tile([NUM_VARS, dma_tile], mybir.dt.float32)
                nc.sync.dma_start(x_sb[:], xt[:, t * dma_tile:(t + 1) * dma_tile])
                for c in range(dma_tile // out_chunk):
                    z_sb = zp.tile([NUM_VARS, out_chunk], mybir.dt.float32)
                    for k in range(out_chunk // mm_n):
                        xsl = slice(c * out_chunk + k * mm_n,
                                    c * out_chunk + (k + 1) * mm_n)
                        zsl = slice(k * mm_n, (k + 1) * mm_n)
                        if do_mm:
                            ps = pp.tile([NUM_VARS, mm_n], mybir.dt.float32)
                            nc.tensor.matmul(
                                ps[:], w_sb[:], x_sb[:, xsl], start=True, stop=True,
                            )
                        if do_mm and do_copy:
                            if copy_split:
                                h = mm_n // 2
                                nc.scalar.copy(
                                    z_sb[:, zsl.start:zsl.start + h], ps[:, :h])
                                nc.vector.tensor_copy(
                                    z_sb[:, zsl.start + h:zsl.stop], ps[:, h:])
                            else:
                                eng = copy_engines[nmm % len(copy_engines)]
                                if eng == "a":
                                    nc.scalar.copy(z_sb[:, zsl], ps[:])
                                else:
                                    nc.vector.tensor_copy(z_sb[:, zsl], ps[:])
                            nmm += 1
                        elif do_out:
                            # plumb a dep so the out DMA still waits on something
                            nc.vector.tensor_copy(z_sb[:, zsl.start:zsl.start + 1],
                                                  x_sb[:, xsl.start:xsl.start + 1])
                    if do_out:
                        out_eng = nc.scalar if out_on_act else nc.sync
                        out_eng.dma_start(
                            zt[:, t * dma_tile + c * out_chunk:
                                  t * dma_tile + (c + 1) * out_chunk],
                            z_sb[:],
                        )
    nc.compile()
    return nc


_CACHE = {}


def kernel(x, weight, mask):
    from concourse.bass_utils import run_bass_kernel_spmd

    x = np.asarray(x, dtype=np.float32)
    weight = np.asarray(weight, dtype=np.float32)
    mask = np.asarray(mask, dtype=np.float32)

    w_eff = _w_eff(weight, mask)
    if "nc" not in _CACHE:
        _CACHE["nc"] = _build_bass()
    nc = _CACHE["nc"]

    xt_full = np.ascontiguousarray(x.T)  # [128, BATCH]
    in_maps = [
        {
            "xt": np.ascontiguousarray(xt_full[:, c * SHARD:(c + 1) * SHARD]),
            "w": w_eff,
        }
        for c in range(N_CORES)
    ]
    res = run_bass_kernel_spmd(nc, in_maps, core_ids=list(range(N_CORES)))
    zt = np.concatenate([r["zt"] for r in res.results], axis=1)  # [128, BATCH]
    return np.ascontiguousarray(zt.T)



# revision 40
# speedup vs baseline: 1.9819x; 1.9819x over previous
"""Trainium2 Bass kernel for nn_LinearSEM.

Reference computes: z = solve_triangular(I - strict_lower(tril(w*mask)), (x*diag)^T).T
Algebraic reformulation: z = x @ W_eff with W_eff = diag(d) @ inv(I-L)^T —
the tiny 128x128 unit-lower-triangular inverse is computed on host in float64
(forward substitution, exact structure, no pivoting noise), and the device
kernel becomes a pure streaming GEMM, which is HBM-bandwidth-bound.

Since the kernel is HBM-traffic-bound, x and z move over HBM as bfloat16
(measured end-to-end rel err ~2.8e-3, well under the 2e-2 gate; fp16/fp8
overflow — z absmax ~3e17). The matmul accumulates in fp32 PSUM; the
PSUM->SBUF copy converts fp32->bf16 for the out-DMA. Host casts x->bf16 and
z(bf16)->fp32 around the device call.

Sharding: data-parallel over batch across 8 cores. Each core receives its
x-shard pre-transposed ([128 vars, 65536 batch]) so the contraction dim (vars)
lands on SBUF partitions; W_eff is the PE-stationary operand (loaded once) and
x^T streams through 512 columns per matmul producing z^T, which is stored
contiguously and un-transposed on host.
"""

import numpy as np

NUM_VARS = 128
BATCH = 524288
N_CORES = 8
SHARD = BATCH // N_CORES  # 65536
DMA_TILE = 4096           # bf16 batch-cols per in-DMA tile: 128p x 8KB = 1 MiB/transfer
MM_N = 512                # max fp32-PSUM moving free dim per matmul
OUT_CHUNK = 2048          # batch-cols per out-DMA: 128p x 4KB = 512 KiB/transfer
K_ZHI = 32                # z rows (by permuted norm rank) kept in bf16; rest fp8
                          # (32, not 16: PSUM partition APs need 32-row alignment)
K_XHI = 32                # x vars (by W_eff row-norm rank) kept in bf16; the rest
K_XLO = NUM_VARS - K_XHI  # stream as fp8e4 (W rows clipped to +-224 on host)


def _w_eff(weight: np.ndarray, mask: np.ndarray) -> np.ndarray:
    n = NUM_VARS
    wl = np.tril(weight.astype(np.float64) * mask.astype(np.float64))
    d = np.diag(wl).copy()
    L = wl - np.diag(d)
    # X = inv(I - L) by forward substitution in float64: X[i,:] = e_i + L[i,:i] @ X[:i,:]
    X = np.eye(n, dtype=np.float64)
    for i in range(1, n):
        X[i, :] += L[i, :i] @ X[:i, :]
    w_eff = d[:, None] * X.T
    return np.ascontiguousarray(w_eff.astype(np.float32))


def _build_bass(
    dma_tile=DMA_TILE,
    mm_n=MM_N,
    out_chunk=OUT_CHUNK,
    xbufs=4,
    zbufs=8,
    pbufs=8,
    copy_engines="av",  # 'a'=ACT only, 'v'=DVE only, 'av'=alternate
    in_queues="s",      # HWDGE queues for in-DMA: s=SP(sync), v=DVE, a=ACT, t=PE, g=GPSIMD
    out_queues="a",     # HWDGE queues for out-DMA
    zsplit=None,        # None | "swdge" | "copies" — fp8 low-norm z rows
    xsplit=False,       # split x: hi rows bf16 + lo rows fp8, 2 acc. matmuls
    xsplit_mm="both",   # diag: "both" | "lo" | "hi" — which matmuls to emit
    xorder="chunk",     # "alt": lo,hi per position; "chunk": all-lo then all-hi
                        # per out_chunk (stationary reloads amortized)
    xlo_dr=True,        # lo matmul in DoubleRow fp8 mode (2 K-rows/cell/cycle);
                        # plain fp8 matmul measures ~2x slower than bf16
    wswitch=2,          # number of identical stationary copies to rotate
                        # through (2 measurably ~3% faster than 1: back-to-back
                        # ldweights from ONE SBUF tile serializes the PE)
    do_mm=True,         # diagnostics: disable stages to isolate bottleneck
    do_copy=True,
    do_out=True,
    reps=1,             # repeat the whole sweep (for slope-based HW timing)
    hwloop=False,       # wrap the sweep in tc.For_i(0, reps) instead of
                        # unrolling: constant instruction count, so huge rep
                        # counts (512+) are cheap to build — used for timing
):
    import concourse.bacc as bacc
    import concourse.mybir as mybir
    from concourse.tile import TileContext

    assert dma_tile % out_chunk == 0 and out_chunk % mm_n == 0
    if xsplit and xorder == "chunk":
        # chunk mode opens out_chunk//mm_n PSUM groups at once under distinct
        # tile names; scale down the per-name buf count to stay in 8 banks
        pbufs = max(1, pbufs // (out_chunk // mm_n))

    nc = bacc.Bacc(None, target_bir_lowering=False)
    bf16 = mybir.dt.bfloat16
    f8 = mybir.dt.float8e4
    if xsplit:
        xh = nc.dram_tensor("xh", [K_XHI, SHARD], bf16, kind="ExternalInput")
        wh = nc.dram_tensor("wh", [K_XHI, NUM_VARS], bf16, kind="ExternalInput")
        if xlo_dr:
            xl = nc.dram_tensor("xl", [K_XLO // 2, 2, SHARD], f8,
                                kind="ExternalInput")
            wl = nc.dram_tensor("wl", [K_XLO // 2, 2, NUM_VARS], f8,
                                kind="ExternalInput")
        else:
            xl = nc.dram_tensor("xl", [K_XLO, SHARD], f8, kind="ExternalInput")
            wl = nc.dram_tensor("wl", [K_XLO, NUM_VARS], f8, kind="ExternalInput")
    else:
        xt = nc.dram_tensor("xt", [NUM_VARS, SHARD], bf16, kind="ExternalInput")
        w = nc.dram_tensor("w", [NUM_VARS, NUM_VARS], bf16, kind="ExternalInput")
    if zsplit is None:
        zt = nc.dram_tensor("zt", [NUM_VARS, SHARD], bf16, kind="ExternalOutput")
    else:
        n_lo = NUM_VARS - K_ZHI
        zt8 = nc.dram_tensor("zt8", [n_lo, SHARD], f8, kind="ExternalOutput")
        zt16 = nc.dram_tensor("zt16", [K_ZHI, SHARD], bf16, kind="ExternalOutput")
        sc = nc.dram_tensor("sc", [NUM_VARS, 1], mybir.dt.float32,
                            kind="ExternalInput")

    def dma_eng(ch):
        return {"s": nc.sync, "v": nc.vector, "a": nc.scalar,
                "t": nc.tensor, "g": nc.gpsimd}[ch]

    with TileContext(nc) as tc:
        with (
            tc.tile_pool(name="wp", bufs=1) as wp,
            tc.tile_pool(name="xp", bufs=xbufs) as xp,
            tc.tile_pool(name="zp", bufs=zbufs) as zp,
            tc.tile_pool(name="pp", bufs=pbufs, space="PSUM") as pp,
        ):
            if xsplit:
                wh_sbs, wl_sbs = [], []
                for i in range(wswitch):
                    whs = wp.tile([K_XHI, NUM_VARS], bf16, name=f"wh{i}")
                    if xlo_dr:
                        wls = wp.tile([K_XLO // 2, 2, NUM_VARS], f8,
                                      name=f"wl{i}")
                    else:
                        wls = wp.tile([K_XLO, NUM_VARS], f8, name=f"wl{i}")
                    nc.sync.dma_start(whs[:], wh[:])
                    nc.sync.dma_start(wls[:], wl[:])
                    wh_sbs.append(whs)
                    wl_sbs.append(wls)
            else:
                w_sbs = []
                for i in range(wswitch):
                    ws = wp.tile([NUM_VARS, NUM_VARS], bf16, name=f"w{i}")
                    nc.sync.dma_start(ws[:], w[:])
                    w_sbs.append(ws)
            if zsplit is not None:
                sc_sb = wp.tile([NUM_VARS, 1], mybir.dt.float32)
                nc.sync.dma_start(sc_sb[:], sc[:])
            nmm = 0
            ndma_in = 0
            ndma_out = 0

            def tile_body(t):
                nonlocal nmm, ndma_in, ndma_out

                def lo_matmul(ps, xsl, start, stop, i=0):
                    wl_sb = wl_sbs[i % wswitch]
                    if xlo_dr:
                        nc.tensor.matmul(
                            ps[:], wl_sb[:], xl_sb[:, :, xsl],
                            perf_mode=mybir.MatmulPerfMode.DoubleRow,
                            start=start, stop=stop)
                    else:
                        nc.tensor.matmul(ps[:], wl_sb[:], xl_sb[:, xsl],
                                         start=start, stop=stop)

                if xsplit:
                    xh_sb = xp.tile([K_XHI, dma_tile], bf16)
                    ts = slice(t * dma_tile, (t + 1) * dma_tile)
                    q = dma_eng(in_queues[ndma_in % len(in_queues)])
                    q.dma_start(xh_sb[:], xh[:, ts])
                    if xlo_dr:
                        xl_sb = xp.tile([K_XLO // 2, 2, dma_tile], f8)
                        q.dma_start(xl_sb[:], xl[:, :, ts])
                    else:
                        xl_sb = xp.tile([K_XLO, dma_tile], f8)
                        q.dma_start(xl_sb[:], xl[:, ts])
                else:
                    x_sb = xp.tile([NUM_VARS, dma_tile], bf16)
                    dma_eng(in_queues[ndma_in % len(in_queues)]).dma_start(
                        x_sb[:], xt[:, t * dma_tile:(t + 1) * dma_tile])
                ndma_in += 1
                for c in range(dma_tile // out_chunk):
                    cols = slice(t * dma_tile + c * out_chunk,
                                 t * dma_tile + (c + 1) * out_chunk)
                    if zsplit == "copies":
                        z8_sb = zp.tile([NUM_VARS - K_ZHI, out_chunk], f8)
                        z16_sb = zp.tile([K_ZHI, out_chunk], bf16)
                    else:
                        z_sb = zp.tile([NUM_VARS, out_chunk], bf16)
                    nk = out_chunk // mm_n
                    pss = [None] * nk
                    if do_mm and xsplit and xorder == "chunk":
                        # batch matmuls by stationary operand so the PE weight
                        # load amortizes over nk positions (one open PSUM
                        # accumulation group per position)
                        for k in range(nk):
                            pss[k] = pp.tile([NUM_VARS, mm_n], mybir.dt.float32,
                                             name=f"ps{k}")
                        for k in range(nk):
                            xsl = slice(c * out_chunk + k * mm_n,
                                        c * out_chunk + (k + 1) * mm_n)
                            if xsplit_mm in ("both", "lo"):
                                lo_matmul(pss[k], xsl,
                                          True, xsplit_mm == "lo", i=k)
                        for k in range(nk):
                            xsl = slice(c * out_chunk + k * mm_n,
                                        c * out_chunk + (k + 1) * mm_n)
                            if xsplit_mm in ("both", "hi"):
                                nc.tensor.matmul(
                                    pss[k][:], wh_sbs[k % wswitch][:],
                                    xh_sb[:, xsl],
                                    start=(xsplit_mm == "hi"), stop=True)
                    for k in range(out_chunk // mm_n):
                        xsl = slice(c * out_chunk + k * mm_n,
                                    c * out_chunk + (k + 1) * mm_n)
                        zsl = slice(k * mm_n, (k + 1) * mm_n)
                        if do_mm:
                            if pss[k] is not None:
                                ps = pss[k]
                            else:
                                ps = pp.tile([NUM_VARS, mm_n], mybir.dt.float32)
                                if xsplit:
                                    if xsplit_mm in ("both", "lo"):
                                        lo_matmul(ps, xsl,
                                                  True, xsplit_mm == "lo", i=k)
                                    if xsplit_mm in ("both", "hi"):
                                        nc.tensor.matmul(
                                            ps[:], wh_sbs[k % wswitch][:],
                                            xh_sb[:, xsl],
                                            start=(xsplit_mm == "hi"), stop=True)
                                else:
                                    nc.tensor.matmul(
                                        ps[:], w_sbs[k % wswitch][:],
                                        x_sb[:, xsl],
                                        start=True, stop=True,
                                    )
                        if do_mm and do_copy:
                            if zsplit == "copies":
                                n_lo = NUM_VARS - K_ZHI
                                # scaled fp8 copy of low rows + bf16 copy of hi
                                # rows, engines alternating per position
                                if nmm % 2 == 0:
                                    nc.scalar.mul(z8_sb[:, zsl], ps[:n_lo, :],
                                                  sc_sb[:n_lo, :])
                                    nc.vector.tensor_copy(z16_sb[:, zsl],
                                                          ps[n_lo:, :])
                                else:
                                    nc.vector.tensor_scalar_mul(
                                        z8_sb[:, zsl], ps[:n_lo, :],
                                        sc_sb[:n_lo, :])
                                    nc.scalar.copy(z16_sb[:, zsl], ps[n_lo:, :])
                            elif zsplit == "swdge":
                                # single scaled bf16 copy; cast happens in DMA
                                eng = copy_engines[nmm % len(copy_engines)]
                                if eng == "a":
                                    nc.scalar.mul(z_sb[:, zsl], ps[:], sc_sb[:])
                                else:
                                    nc.vector.tensor_scalar_mul(
                                        z_sb[:, zsl], ps[:], sc_sb[:])
                            else:
                                eng = copy_engines[nmm % len(copy_engines)]
                                if eng == "a":
                                    nc.scalar.copy(z_sb[:, zsl], ps[:])
                                else:
                                    nc.vector.tensor_copy(z_sb[:, zsl], ps[:])
                            nmm += 1
                        elif do_out:
                            # plumb a dep so the out DMA still waits on something
                            nc.vector.tensor_copy(z_sb[:, zsl.start:zsl.start + 1],
                                                  x_sb[:, xsl.start:xsl.start + 1])
                    if do_out:
                        oq = dma_eng(out_queues[ndma_out % len(out_queues)])
                        if zsplit == "copies":
                            n_lo = NUM_VARS - K_ZHI
                            oq.dma_start(zt8[:, cols], z8_sb[:])
                            nc.sync.dma_start(zt16[:, cols], z16_sb[:])
                        elif zsplit == "swdge":
                            n_lo = NUM_VARS - K_ZHI
                            nc.gpsimd.dma_start(zt8[:, cols], z_sb[:n_lo, :])
                            oq.dma_start(zt16[:, cols], z_sb[n_lo:, :])
                        else:
                            oq.dma_start(zt[:, cols], z_sb[:])
                        ndma_out += 1

            ntiles = SHARD // dma_tile
            if hwloop:
                with tc.For_i(0, reps):
                    for t in range(ntiles):
                        tile_body(t)
            else:
                for t in range(reps * ntiles):
                    tile_body(t % ntiles)
    nc.compile()
    return nc


_CACHE = {}


def kernel(x, weight, mask):
    import ml_dtypes
    from concourse.bass_utils import run_bass_kernel_spmd

    x = np.asarray(x, dtype=np.float32)
    weight = np.asarray(weight, dtype=np.float32)
    mask = np.asarray(mask, dtype=np.float32)

    w_eff = _w_eff(weight, mask).astype(ml_dtypes.bfloat16)
    if "nc" not in _CACHE:
        _CACHE["nc"] = _build_bass()
    nc = _CACHE["nc"]

    xt_full = np.ascontiguousarray(x.astype(ml_dtypes.bfloat16).T)  # [128, BATCH] bf16
    in_maps = [
        {
            "xt": np.ascontiguousarray(xt_full[:, c * SHARD:(c + 1) * SHARD]),
            "w": w_eff,
        }
        for c in range(N_CORES)
    ]
    res = run_bass_kernel_spmd(nc, in_maps, core_ids=list(range(N_CORES)))
    zt = np.concatenate([r["zt"] for r in res.results], axis=1)  # [128, BATCH] bf16
    return np.ascontiguousarray(zt.T.astype(np.float32))
